# revision 22
# baseline (speedup 1.0000x reference)
"""Trainium2 Bass kernel for leave-one-out Nadaraya-Watson regression
(nn_Net_72877005078649) — fast-Gauss-transform (Taylor moment) algorithm.

Math:
  Xw = mlp(train_X) [N,10], Zw = mlp(x) [B,10]  (mlp = W2 @ relu(W1 @ .))
  K[b,n,o] = exp(-0.5*((Xw[n,o]-Zw[b,o])/h)^2), K[b,b,:] = 0
  out[b,o] = sum_n K*Y[n,o] / sum_n K

Key reformulation (x' = Xw/h, z' = Zw/h):
  K = e^{-x'^2/2} * e^{x' z'} * e^{-z'^2/2}; the last factor is constant
  over n and cancels in the num/den ratio.  Expanding e^{x'z'} in a
  KT-term Taylor series collapses the O(B*N*O) kernel sum to per-channel
  moments:
    num[b,o] = sum_k z'^k/k! * M_k,o,   M_k,o = sum_n Y[n,o] e^{-x'^2/2} x'^k
  (den likewise with Y:=1).  max |x' z'| ~ 4.7 on this data; KT=12 terms
  give rel err ~3e-4 end-to-end (measured on HW) vs the 2e-2 gate —
  per-element truncation error is diluted by the 4096-term positive n-sum.

Device pipeline per core (B sharded 8 ways -> 512 queries/core; N, Y, W
replicated; no collectives):
  inputs arrive HOST-pre-transposed and bf16: xT [64, (tX|xq|td) rows],
  weights first, tXT staggered (512/1024/1024/1536 cols) across 2 DMA
  queues so MM1 starts on the first piece
  4 bf16 K=128 dummy matmuls during the input DMAs start tripping the
  PE HAM clock gate 1.2 -> 2.4 GHz (HAM ignores K<128 and fp32 matmuls)
  MM1 (bf16, stationary w1T) + relu (alternating ACT/DVE) -> H bf16;
  MM2 per 128-row tile (lhsT=H tile) -> X' = Xw/h in one PSUM bank
  the rest runs in 2 half-blocks so the DVE power chain of half A
  overlaps MM2s of half B, and moment matmuls overlap the next chain:
    d = exp(-x'^2/2) (ACT), c = Y*d (GPSIMD) -> cd [128,(32,2,10)] f32r
    power table V[128,(36,10,12)] f32r by 11 serial DVE multiplies
    (query powers are chunks 32-35 of the same table — same recurrence)
    moments: 32 accumulated matmuls lhsT=cd[128,20] rhs=V[128,120] ->
      PSUM [20,(o,k)]; only the o==o' diagonal blocks are used
  diag-select + 1/k!-scale via host-built masks (2 DVE mults), then a
    ones-weighted matmul broadcasts M to all 128 partitions
  eval num/den = one DVE mult over all 4 query tiles (M2 stride-0
    broadcast) + one tensor_reduce over k each, exact leave-one-out
    diagonal subtraction from the td/yd projections, divide, DMA out.
"""

import numpy as np

N = 4096
D = 64
HID = 128
O = 10
NCORES = 8
BQ = N // NCORES          # queries per core (512)
KT = 12                   # Taylor terms
NT_SRC = N // 128         # 32 source tiles
NTILES = NT_SRC + 8       # + 4 query tiles + 4 diag tiles
NPOW = NT_SRC + 4         # chunks carrying power tables (src + query)
NQUAD = NTILES // 4       # 10 quads of 512 rows

_cache = {}


def _build(h: float):
    import concourse.bass as bass
    import concourse.bacc as bacc
    import concourse.tile as tile
    from concourse import mybir

    f32 = mybir.dt.float32
    f32r = mybir.dt.float32r
    bf16 = mybir.dt.bfloat16
    AF = mybir.ActivationFunctionType
    ALU = mybir.AluOpType

    nc = bacc.Bacc("TRN2", target_bir_lowering=False, debug=False, num_devices=1)
    xqT = nc.dram_tensor("xqT", [D, BQ], bf16, kind="ExternalInput").ap()
    tXT = nc.dram_tensor("tXT", [D, N], bf16, kind="ExternalInput").ap()
    tdT = nc.dram_tensor("tdT", [D, BQ], bf16, kind="ExternalInput").ap()
    w1T = nc.dram_tensor("w1T", [D, HID], bf16, kind="ExternalInput").ap()
    w2T = nc.dram_tensor("w2T", [HID, O], bf16, kind="ExternalInput").ap()
    Yt = nc.dram_tensor("Y", [N, O], bf16, kind="ExternalInput").ap()
    yd = nc.dram_tensor("yd", [BQ, O], bf16, kind="ExternalInput").ap()
    EJ = nc.dram_tensor("EJ", [20, 2 * O * KT], f32, kind="ExternalInput").ap()
    out = nc.dram_tensor("out", [BQ, O], f32, kind="ExternalOutput").ap()

    with tile.TileContext(nc) as tc:
        with (
            tc.tile_pool(name="singles", bufs=1) as S,
            tc.tile_pool(name="work", bufs=3) as W,
            tc.tile_pool(name="psW", bufs=1, space="PSUM") as PSW,
            tc.tile_pool(name="psH", bufs=3, space="PSUM") as PSH,
            tc.tile_pool(name="psX", bufs=1, space="PSUM") as PSX,
            tc.tile_pool(name="psM", bufs=1, space="PSUM") as PSM,
        ):
            # ---------------- constants ----------------
            warm = S.tile([1, 16], f32)
            nc.vector.memset(warm, 0.0)
            nc.scalar.activation(out=warm, in_=warm, func=AF.Exp)
            ones128 = S.tile([128, 512], bf16)
            nc.vector.memset(ones128, 1.0)
            ones32f = S.tile([20, 128], f32)
            nc.vector.memset(ones32f, 1.0)
            onesW = S.tile([20, 128], f32r)
            nc.vector.tensor_copy(onesW, ones32f)
            vones = S.tile([128, NPOW * O], f32)
            nc.vector.memset(vones, 1.0)

            # PE HAM warm-up: K=128 bf16 matmuls while inputs stream in
            wps = PSW.tile([128, 512], f32, tag="warm", name="wps")
            for i in range(4):
                nc.tensor.matmul(wps, lhsT=ones128[:, 0:128], rhs=ones128,
                                 start=True, stop=True)

            # -------- input DMAs: weights first, tXT staggered --------
            w1sb = S.tile([D, HID], bf16)
            nc.sync.dma_start(out=w1sb, in_=w1T)
            w2sb = S.tile([HID, O], bf16)
            nc.sync.dma_start(out=w2sb, in_=w2T)
            xT = S.tile([D, NTILES * 128], bf16)
            cuts = [0, 512, 1536, 2560, N]
            for i in range(4):
                eng = nc.sync if i % 2 == 0 else nc.gpsimd
                eng.dma_start(out=xT[:, cuts[i]:cuts[i + 1]],
                              in_=tXT[:, cuts[i]:cuts[i + 1]])
            nc.sync.dma_start(out=xT[:, N:N + BQ], in_=xqT)
            nc.gpsimd.dma_start(out=xT[:, N + BQ:N + 2 * BQ], in_=tdT)
            Ej = S.tile([20, 2 * O * KT], f32)
            nc.scalar.dma_start(out=Ej, in_=EJ)
            Ytab = S.tile([128, NT_SRC * O], bf16)
            nc.scalar.dma_start(out=Ytab.rearrange("p (t o) -> p t o", o=O),
                                in_=Yt.rearrange("(t p) o -> p t o", p=128))
            ydT = S.tile([128, 4 * O], bf16)
            nc.scalar.dma_start(out=ydT.rearrange("p (t o) -> p t o", o=O),
                                in_=yd.rearrange("(t p) o -> p t o", p=128))

            # ---------------- MM1 + relu -> H ----------------
            H = S.tile([128, NTILES * 128], bf16)
            for q in range(NQUAD):
                hp = PSH.tile([128, 512], f32, tag="H", name="hps")
                nc.tensor.matmul(hp, lhsT=w1sb,
                                 rhs=xT[:, q * 512:(q + 1) * 512],
                                 start=True, stop=True)
                dst = H[:, q * 512:(q + 1) * 512]
                if q % 2 == 0:
                    nc.scalar.activation(out=dst, in_=hp, func=AF.Relu)
                else:
                    nc.vector.tensor_scalar_max(dst, hp, 0.0)

            # ---------- MM2 -> X', then the two half-blocks ----------
            xps = PSX.tile([128, NTILES * O], f32, tag="xp", name="xps")
            Xp = S.tile([128, NTILES * O], f32)
            sq = S.tile([128, NTILES * O], f32)
            cd = S.tile([128, NT_SRC * 2 * O], f32r)
            cd4 = cd.rearrange("p (c j o) -> p c j o", j=2, o=O)
            sq4 = sq.rearrange("p (c o) -> p c o", o=O)
            Yt4 = Ytab.rearrange("p (c o) -> p c o", o=O)
            V = S.tile([128, NPOW * O * KT], f32r)
            V4 = V.rearrange("p (c o k) -> p c o k", o=O, k=KT)
            vo4 = vones.rearrange("p (c o) -> p c o", o=O)
            Xs4 = Xp.rearrange("p (c o) -> p c o", o=O)
            mps = PSM.tile([20, O * KT], f32, tag="M", name="mps")

            QC = NT_SRC * O          # col offset of query block (320)
            DC = (NT_SRC + 4) * O    # col offset of diag block (360)

            def mm2(ts):
                for t in ts:
                    nc.tensor.matmul(
                        xps[:, t * O:(t + 1) * O],
                        lhsT=H[:, t * 128:(t + 1) * 128], rhs=w2sb,
                        start=True, stop=True)

            def halfblock(hb):
                # half A: chunks 0-15; half B: chunks 16-31 src + 32-35 query
                c0, c1 = (0, 16) if hb == 0 else (16, NPOW)
                cs = slice(c0, c1)
                csrc = slice(c0, min(c1, NT_SRC))
                x0, x1 = c0 * O, c1 * O
                xe = x1 if hb == 0 else NTILES * O   # incl diag cols in B
                nc.scalar.copy(Xp[:, x0:xe], xps[:, x0:xe])
                nc.scalar.square(sq[:, x0:xe], Xp[:, x0:xe])
                nc.scalar.activation(out=cd4[:, csrc, 1, :], in_=sq4[:, csrc, :],
                                     func=AF.Exp, scale=-0.5)
                nc.gpsimd.tensor_mul(cd4[:, csrc, 0, :], Yt4[:, csrc, :],
                                     cd4[:, csrc, 1, :])
                nc.vector.tensor_copy(V4[:, cs, :, 0], vo4[:, cs, :])
                for k in range(1, KT):
                    nc.vector.tensor_mul(V4[:, cs, :, k], V4[:, cs, :, k - 1],
                                         Xs4[:, cs, :])

            def moments(hb):
                c0, c1 = (0, 16) if hb == 0 else (16, NT_SRC)
                for c in range(c0, c1):
                    nc.tensor.matmul(
                        mps, lhsT=cd[:, c * 2 * O:(c + 1) * 2 * O],
                        rhs=V[:, c * O * KT:(c + 1) * O * KT],
                        start=(c == 0), stop=(c == NT_SRC - 1))

            mm2(range(0, 16))
            halfblock(0)
            mm2(range(16, NTILES))       # PE continues before momentsA
            moments(0)
            halfblock(1)
            moments(1)

            # select diag blocks M[j*10+o, (o,k)] (1/k! in the mask) and
            # broadcast to 128 partitions via a ones-weighted matmul
            masked = S.tile([20, 2 * O * KT], f32r)
            nc.vector.tensor_mul(masked[:, 0:O * KT], mps, Ej[:, 0:O * KT])
            nc.vector.tensor_mul(masked[:, O * KT:], mps, Ej[:, O * KT:])
            m2ps = PSX.tile([128, 2 * O * KT], f32, tag="m2", name="m2ps")
            nc.tensor.matmul(m2ps, lhsT=onesW, rhs=masked, start=True, stop=True)
            M2 = S.tile([128, 2 * O * KT], f32)
            nc.scalar.copy(M2, m2ps)

            # ---------------- eval ----------------
            num = S.tile([128, 4 * O], f32)
            den = S.tile([128, 4 * O], f32)
            M2P = M2.ap[0][0]
            UQ = V[:, NT_SRC * O * KT:NPOW * O * KT]   # query powers
            for j, acc in ((0, num), (1, den)):
                m2b = bass.AP(tensor=M2.tensor, offset=M2.offset + j * O * KT,
                              ap=[[M2P, 128], [0, 4], [1, O * KT]])
                p1 = W.tile([128, 4 * O * KT], f32, tag="p1")
                nc.vector.tensor_mul(
                    p1.rearrange("p (qc f) -> p qc f", f=O * KT),
                    UQ.rearrange("p (qc f) -> p qc f", f=O * KT), m2b)
                nc.vector.tensor_reduce(
                    acc, p1.rearrange("p (qc o k) -> p qc o k", o=O, k=KT),
                    axis=mybir.AxisListType.X, op=ALU.add)

            # ---------------- diagonal correction ----------------
            t1 = S.tile([128, 4 * O], f32)
            nc.vector.tensor_mul(t1, Xp[:, DC:DC + 4 * O], Xp[:, QC:QC + 4 * O])
            nc.vector.scalar_tensor_tensor(
                out=t1, in0=sq[:, DC:DC + 4 * O], scalar=-0.5, in1=t1,
                op0=ALU.mult, op1=ALU.add)
            kd = S.tile([128, 4 * O], f32)
            nc.scalar.activation(out=kd, in_=t1, func=AF.Exp)
            nc.vector.tensor_mul(t1, kd, ydT)
            nc.vector.tensor_sub(num, num, t1)
            nc.vector.tensor_sub(den, den, kd)
            rec = S.tile([128, 4 * O], f32)
            nc.vector.reciprocal(rec, den)
            nc.vector.tensor_mul(num, num, rec)

            nc.sync.dma_start(out=out.rearrange("(c p) o -> p c o", p=128),
                              in_=num.rearrange("p (c o) -> p c o", o=O))

    nc.compile()
    return nc


def _ej_const():
    """[20, (j,o,k)] mask: row j*10+o keeps block (j, o, :) with value 1/k!."""
    ej = np.zeros((20, 2 * O * KT), np.float32)
    fact = np.cumprod(np.concatenate([[1.0], np.arange(1, KT)])).astype(np.float64)
    for j in range(2):
        for o in range(O):
            ej[j * O + o, (j * O + o) * KT:(j * O + o + 1) * KT] = 1.0 / fact
    return ej


def make_in_maps(x, train_X, Y, W1, W2, h):
    import ml_dtypes
    bf = ml_dtypes.bfloat16
    x = np.ascontiguousarray(x, dtype=np.float32)
    train_X = np.ascontiguousarray(train_X, dtype=np.float32)
    Yb = np.ascontiguousarray(Y).astype(bf)
    tXT = np.ascontiguousarray(train_X.T).astype(bf)
    w1t = np.ascontiguousarray(np.asarray(W1, np.float32).T).astype(bf)
    w2t = np.ascontiguousarray((np.asarray(W2, np.float32) / float(h)).T).astype(bf)
    ej = _ej_const()
    in_maps = []
    for c in range(NCORES):
        sl = slice(c * BQ, (c + 1) * BQ)
        in_maps.append({
            "xqT": np.ascontiguousarray(x[sl].T).astype(bf),
            "tXT": tXT,
            "tdT": np.ascontiguousarray(train_X[sl].T).astype(bf),
            "w1T": w1t, "w2T": w2t,
            "Y": Yb, "yd": Yb[sl], "EJ": ej,
        })
    return in_maps


def kernel(x, train_X, Y, W1, W2, h):
    import concourse.bass_utils as bass_utils

    hval = float(h)
    key = ("fgt5", hval)
    if key not in _cache:
        _cache[key] = _build(hval)
    nc = _cache[key]

    in_maps = make_in_maps(x, train_X, Y, W1, W2, hval)
    res = bass_utils.run_bass_kernel_spmd(nc, in_maps, core_ids=list(range(NCORES)))
    return np.concatenate([res.results[c]["out"] for c in range(NCORES)], axis=0)
